# revision 9
# baseline (speedup 1.0000x reference)
# Trainium2 Bass kernel for AudioContextAggregation (windowed cross-attention).
#
# Strategy: data-parallel over batch B=16 across 8 NeuronCores (2 batches/core).
# Host folds LayerNorm gamma/beta and the 1/sqrt(hd) attention scale into the
# projection weights/biases (shipped as bf16), zero-pads audio along the frame
# axis so the per-batch window gather becomes a single dynamically-offset DMA,
# and pre-transposes all weights to the K-major layout the PE array wants.
# Device pipeline per batch:
#   kv gather -> LN -> kvT -> K/V projections (tiny: 50 tokens)
#   per 512-token chunk of the 1024 queries:
#     LN(hidden) -> xhatT (PE transpose) -> Q proj -> scores -> softmax ->
#     probsT (PE transpose) -> ctx -> out proj (+bias via K=1 ones matmul)
#     -> +hidden residual in fp32 -> DMA out
# All matmuls run in bf16 (1 cycle/row on TRN2); LN stats, softmax sums and the
# residual add stay fp32.

import numpy as np
import ml_dtypes

import concourse.bass as bass
import concourse.mybir as mybir
import concourse.tile as tile
from concourse import bacc
from concourse.bass import ds
from concourse.bass_utils import run_bass_kernel_spmd
from concourse.masks import make_identity

F32 = mybir.dt.float32
BF16 = mybir.dt.bfloat16
I32 = mybir.dt.int32
AF = mybir.ActivationFunctionType
ALU = mybir.AluOpType
AX = mybir.AxisListType

NCORES = 8
B, T, L, D = 16, 200, 5, 1024
LQ = 1024
H, HD = 8, 128
NB = B // NCORES            # batches per core
WIN_LO, WIN_HI = -4, 6
WIN = WIN_HI - WIN_LO       # 10 frames
KV = WIN * L                # 50 kv tokens
PAD_LO = -WIN_LO            # 4 zero frames in front
PADF = PAD_LO + T + WIN_HI  # 210 padded frames
PADR = PADF * L             # 1050 padded rows
KC = D // 128               # 8 feature chunks of 128
TCH = 512                   # token chunk (matmul N)
NCH = LQ // TCH             # chunks per batch
TPC = TCH // 128            # 128-token tiles per chunk
EPS = 1e-5

# set False to replace the broadcast tensor_tensor softmax-normalize with a
# per-head tensor_scalar loop
BCAST_NORM = True


def _bcast_last(ap: bass.AP, n: int) -> bass.AP:
    """Append a stride-0 trailing dim of size n to an AP (free-axis bcast)."""
    return bass.AP(tensor=ap.tensor, offset=ap.offset, ap=ap.ap + [[0, n]])


def _layernorm_stats(nc, pools, x_ap, p):
    """Emit LN stats for x_ap [p, D] (f32). Returns (rstd, neg_mu_rstd) [p,1]."""
    st = pools["stats"]
    nsub = D // 512
    stats = st.tile([128, nsub, 6], F32, tag="bnst")
    xg = x_ap.rearrange("p (s d) -> p s d", s=nsub)
    for s in range(nsub):
        nc.vector.bn_stats(out=stats[:p, s, :], in_=xg[:, s, :])
    mv = st.tile([128, 2], F32, tag="bnmv")
    nc.vector.bn_aggr(out=mv[:p], in_=stats[:p])
    rstd = st.tile([128, 1], F32, tag="rstd")
    # rstd = 1/sqrt(var + eps)
    nc.scalar.activation(out=rstd[:p], in_=mv[:p, 1:2], func=AF.Sqrt,
                         bias=pools["eps"][:p], scale=1.0)
    nc.vector.reciprocal(out=rstd[:p], in_=rstd[:p])
    nmr = st.tile([128, 1], F32, tag="nmr")
    # nmr = -(mean * rstd)
    nc.vector.tensor_scalar(out=nmr[:p], in0=mv[:p, 0:1], scalar1=rstd[:p],
                            scalar2=-1.0, op0=ALU.mult, op1=ALU.mult)
    return rstd, nmr


def build_program() -> bass.Bass:
    nc = bacc.Bacc("TRN2", target_bir_lowering=False, debug=False)

    hid = nc.declare_dram_parameter("hidden", [NB, LQ, D], F32, isOutput=False)
    aud = nc.declare_dram_parameter("audio_pad", [NB, PADR, D], F32, isOutput=False)
    srow = nc.declare_dram_parameter("start_row", [1, NB], I32, isOutput=False)
    wq = nc.declare_dram_parameter("wq", [128, KC, D], BF16, isOutput=False)
    wk = nc.declare_dram_parameter("wk", [128, KC, D], BF16, isOutput=False)
    wv = nc.declare_dram_parameter("wv", [128, KC, D], BF16, isOutput=False)
    wo = nc.declare_dram_parameter("wo", [128, KC, D], BF16, isOutput=False)
    bq = nc.declare_dram_parameter("bq", [128, KC], F32, isOutput=False)
    bk = nc.declare_dram_parameter("bk", [128, KC], F32, isOutput=False)
    bo = nc.declare_dram_parameter("bo", [1, D], BF16, isOutput=False)
    out = nc.declare_dram_parameter("out", [NB, LQ, D], F32, isOutput=True)

    with tile.TileContext(nc) as tc:
        _emit(tc, hid, aud, srow, wq, wk, wv, wo, bq, bk, bo, out)
    nc.compile()
    return nc


def _emit(tc, hid, aud, srow, wq, wk, wv, wo, bq, bk, bo, out):
    nc = tc.nc

    with (
        tc.tile_pool(name="const", bufs=1) as const,
        tc.tile_pool(name="wpool", bufs=1) as wpool,
        tc.tile_pool(name="xp", bufs=5) as xp,
        tc.tile_pool(name="xhp", bufs=3) as xhp,
        tc.tile_pool(name="stats", bufs=8) as stats,
        tc.tile_pool(name="xtp", bufs=2) as xtp,
        tc.tile_pool(name="qtp", bufs=2) as qtp,
        tc.tile_pool(name="prp", bufs=6) as prp,
        tc.tile_pool(name="ptp", bufs=2) as ptp,
        tc.tile_pool(name="ctp", bufs=2) as ctp,
        tc.tile_pool(name="osp", bufs=2) as osp,
        tc.tile_pool(name="kvp", bufs=1) as kvp,
        tc.tile_pool(name="psum", bufs=8, space="PSUM") as psum,
    ):
        pools = {"stats": stats}

        # ---- constants / weights (one-time) ----
        ident = const.tile([128, 128], BF16, tag="ident")
        make_identity(nc, ident)
        eps_t = const.tile([128, 1], F32, tag="eps")
        nc.vector.memset(eps_t, EPS)
        pools["eps"] = eps_t
        ones_r = const.tile([1, 128], BF16, tag="ones")
        nc.vector.memset(ones_r, 1.0)

        w_sb = {}
        for name, prm in (("wq", wq), ("wk", wk), ("wv", wv), ("wo", wo)):
            t = wpool.tile([128, KC, D], BF16, tag=name)
            nc.sync.dma_start(out=t, in_=prm[:])
            w_sb[name] = t
        bq_sb = const.tile([128, KC], F32, tag="bq")
        nc.gpsimd.dma_start(out=bq_sb, in_=bq[:])
        bk_sb = const.tile([128, KC], F32, tag="bk")
        nc.gpsimd.dma_start(out=bk_sb, in_=bk[:])
        bo_sb = const.tile([1, D], BF16, tag="bo")
        nc.gpsimd.dma_start(out=bo_sb, in_=bo[:])

        srow_sb = const.tile([1, NB], I32, tag="srow")
        nc.gpsimd.dma_start(out=srow_sb, in_=srow[:])
        rows = [
            nc.values_load(srow_sb[0:1, b:b + 1], min_val=0,
                           max_val=(PADF - WIN) * L,
                           skip_runtime_bounds_check=True)
            for b in range(NB)
        ]

        for b in range(NB):
            # ================= KV path (50 tokens) =================
            kv_raw = kvp.tile([KV, D], F32, tag="kv_raw")
            nc.gpsimd.dma_start(out=kv_raw, in_=aud[b, ds(rows[b], KV), :])

            rstd, nmr = _layernorm_stats(nc, pools, kv_raw, KV)
            kvh = kvp.tile([KV, D], BF16, tag="kvh")
            nc.scalar.activation(out=kvh, in_=kv_raw, func=AF.Identity,
                                 bias=nmr[:KV], scale=rstd[:KV])

            # kvT[kc] : [128, KV]  (feature-major)
            kvT = kvp.tile([128, KC, KV], BF16, tag="kvT")
            for kc in range(KC):
                pt = psum.tile([128, 512], BF16, tag="ps")
                nc.tensor.transpose(pt[:, :KV], kvh[:, kc * 128:(kc + 1) * 128],
                                    ident[:KV, :KV])
                nc.vector.tensor_copy(out=kvT[:, kc, :], in_=pt[:, :KV])

            # kT[oc] : [128, KV]  (features on partitions)
            kT = kvp.tile([128, KC, KV], BF16, tag="kT")
            for oc in range(KC):
                pq = psum.tile([128, 512], F32, tag="ps")
                for kc in range(KC):
                    nc.tensor.matmul(pq[:, :KV],
                                     w_sb["wk"][:, kc, oc * 128:(oc + 1) * 128],
                                     kvT[:, kc, :],
                                     start=(kc == 0), stop=(kc == KC - 1))
                nc.scalar.activation(out=kT[:, oc, :], in_=pq[:, :KV],
                                     func=AF.Identity, bias=bk_sb[:, oc:oc + 1],
                                     scale=1.0)

            # v : [KV, D] token-major
            v_sb = kvp.tile([KV, D], BF16, tag="v")
            for vc in range(D // TCH):
                pv = psum.tile([128, 512], F32, tag="ps")
                for kc in range(KC):
                    nc.tensor.matmul(pv[:KV, :], kvT[:, kc, :],
                                     w_sb["wv"][:, kc, vc * TCH:(vc + 1) * TCH],
                                     start=(kc == 0), stop=(kc == KC - 1))
                nc.vector.tensor_copy(out=v_sb[:, vc * TCH:(vc + 1) * TCH],
                                      in_=pv[:KV, :])

            # ================= queries, 512-token chunks =================
            for ch in range(NCH):
                xt_c = xtp.tile([128, KC, TCH], BF16, tag="xt")
                x_tiles = []
                for tt in range(TPC):
                    t0 = ch * TCH + tt * 128
                    x_t = xp.tile([128, D], F32, tag="x")
                    x_tiles.append(x_t)
                    nc.sync.dma_start(out=x_t, in_=hid[b, t0:t0 + 128, :])

                    rstd, nmr = _layernorm_stats(nc, pools, x_t, 128)
                    xh = xhp.tile([128, D], BF16, tag="xh")
                    nc.scalar.activation(out=xh, in_=x_t, func=AF.Identity,
                                         bias=nmr, scale=rstd)
                    # transpose 8 x [128,128], packed 4-per-PSUM-bank
                    for g in range(2):
                        pg = psum.tile([128, 512], BF16, tag="ps")
                        for j in range(4):
                            kc = g * 4 + j
                            nc.tensor.transpose(
                                pg[:, j * 128:(j + 1) * 128],
                                xh[:, kc * 128:(kc + 1) * 128], ident)
                        nc.vector.tensor_copy(
                            out=xt_c[:, g * 4:(g + 1) * 4, tt * 128:(tt + 1) * 128],
                            in_=pg.rearrange("p (j f) -> p j f", j=4))

                # Q projection: qT_c[oc] = [128, TCH]
                qT_c = qtp.tile([128, KC, TCH], BF16, tag="qt")
                for oc in range(KC):
                    pq = psum.tile([128, TCH], F32, tag="ps")
                    for kc in range(KC):
                        nc.tensor.matmul(pq,
                                         w_sb["wq"][:, kc, oc * 128:(oc + 1) * 128],
                                         xt_c[:, kc, :],
                                         start=(kc == 0), stop=(kc == KC - 1))
                    nc.scalar.activation(out=qT_c[:, oc, :], in_=pq,
                                         func=AF.Identity,
                                         bias=bq_sb[:, oc:oc + 1], scale=1.0)

                # scores + softmax per 128-token tile
                pr_tiles = []
                for tt in range(TPC):
                    sc = psum.tile([128, H, KV], F32, tag="ps")
                    for h in range(H):
                        nc.tensor.matmul(sc[:, h, :],
                                         qT_c[:, h, tt * 128:(tt + 1) * 128],
                                         kT[:, h, :], start=True, stop=True)
                    pr = prp.tile([128, H, KV], BF16, tag="pr")
                    nc.scalar.activation(out=pr, in_=sc, func=AF.Exp,
                                         bias=0.0, scale=1.0)
                    ssum = stats.tile([128, H], F32, tag="ssum")
                    nc.vector.reduce_sum(out=ssum, in_=pr, axis=AX.X)
                    rec = stats.tile([128, H], F32, tag="rec")
                    nc.vector.reciprocal(out=rec, in_=ssum)
                    if BCAST_NORM:
                        nc.vector.tensor_mul(out=pr, in0=pr,
                                             in1=_bcast_last(rec[:, :], KV))
                    else:
                        for h in range(H):
                            nc.vector.tensor_scalar_mul(
                                out=pr[:, h, :], in0=pr[:, h, :],
                                scalar1=rec[:, h:h + 1])
                    pr_tiles.append(pr)

                # probsT per head: [KV, TCH]
                probsT = ptp.tile([KV, H, TCH], BF16, tag="pt")
                for h in range(H):
                    pt = psum.tile([KV, TCH], BF16, tag="ps")
                    for tt in range(TPC):
                        nc.tensor.transpose(pt[:, tt * 128:(tt + 1) * 128],
                                            pr_tiles[tt][:, h, :], ident)
                    nc.vector.tensor_copy(out=probsT[:, h, :], in_=pt)

                # ctx^T per head: head h == feature chunk h
                ctxT = ctp.tile([128, KC, TCH], BF16, tag="ct")
                for h in range(H):
                    pc = psum.tile([128, TCH], F32, tag="ps")
                    nc.tensor.matmul(pc, v_sb[:, h * 128:(h + 1) * 128],
                                     probsT[:, h, :], start=True, stop=True)
                    nc.scalar.activation(out=ctxT[:, h, :], in_=pc,
                                         func=AF.Identity, bias=0.0, scale=1.0)

                # out projection + bias + residual, per 128-token tile
                for tt in range(TPC):
                    t0 = ch * TCH + tt * 128
                    osb = osp.tile([128, D], F32, tag="os")
                    for vc in range(D // TCH):
                        po = psum.tile([128, TCH], F32, tag="ps")
                        for kc in range(KC):
                            nc.tensor.matmul(
                                po, ctxT[:, kc, tt * 128:(tt + 1) * 128],
                                w_sb["wo"][:, kc, vc * TCH:(vc + 1) * TCH],
                                start=(kc == 0), stop=False)
                        nc.tensor.matmul(po, ones_r,
                                         bo_sb[:, vc * TCH:(vc + 1) * TCH],
                                         start=False, stop=True)
                        nc.vector.tensor_add(
                            out=osb[:, vc * TCH:(vc + 1) * TCH], in0=po,
                            in1=x_tiles[tt][:, vc * TCH:(vc + 1) * TCH])
                    nc.sync.dma_start(out=out[b, t0:t0 + 128, :], in_=osb)


# ---------------------------------------------------------------------------
# host side
# ---------------------------------------------------------------------------

_PROG = None


def get_program() -> bass.Bass:
    global _PROG
    if _PROG is None:
        _PROG = build_program()
    return _PROG


def prep_inputs(hidden, audio_features, frame_idx, q_gamma, q_beta, kv_gamma,
                kv_beta, in_proj_w, in_proj_b, out_w, out_b):
    """Host-side sharding + parameter folding. Returns list of per-core maps."""
    hidden = np.asarray(hidden, np.float32)
    audio = np.asarray(audio_features, np.float32)
    fidx = np.asarray(frame_idx).astype(np.int64)
    q_gamma = np.asarray(q_gamma, np.float64)
    q_beta = np.asarray(q_beta, np.float64)
    kv_gamma = np.asarray(kv_gamma, np.float64)
    kv_beta = np.asarray(kv_beta, np.float64)
    w_in = np.asarray(in_proj_w, np.float64)
    b_in = np.asarray(in_proj_b, np.float64)
    w_out = np.asarray(out_w, np.float64)
    b_out = np.asarray(out_b, np.float64)

    Wq, Wk, Wv = w_in[:D], w_in[D:2 * D], w_in[2 * D:]
    bqv, bkv, bvv = b_in[:D], b_in[D:2 * D], b_in[2 * D:]
    s = 1.0 / np.sqrt(HD)

    Wq_f = Wq * q_gamma[None, :] * s
    bq_f = (bqv + Wq @ q_beta) * s
    Wk_f = Wk * kv_gamma[None, :]
    bk_f = bkv + Wk @ kv_beta
    Wv_f = Wv * kv_gamma[None, :]
    bv_f = bvv + Wv @ kv_beta
    bo_f = b_out + w_out @ bv_f

    def chunkT(w):  # [o,d] -> wT [d,o] -> [128, KC, D] (p, c, o)
        wt = np.ascontiguousarray(w.T).astype(np.float32)
        return np.ascontiguousarray(
            wt.reshape(KC, 128, D).transpose(1, 0, 2)).astype(ml_dtypes.bfloat16)

    wq_ship = chunkT(Wq_f)
    wk_ship = chunkT(Wk_f)
    wv_ship = chunkT(Wv_f)
    wo_ship = chunkT(w_out)

    def colsplit(bias):  # (D,) -> [128, KC]
        return np.ascontiguousarray(
            bias.astype(np.float32).reshape(KC, 128).T)

    bq_ship = colsplit(bq_f)
    bk_ship = colsplit(bk_f)
    bo_ship = bo_f.astype(np.float32).astype(ml_dtypes.bfloat16).reshape(1, D)

    # zero-padded audio: frame f -> rows (f+PAD_LO)*L ...; window start row =
    # (idx + WIN_LO + PAD_LO) * L = idx * L
    audio_pad = np.zeros((B, PADR, D), np.float32)
    audio_pad[:, PAD_LO * L:(PAD_LO + T) * L, :] = audio.reshape(B, T * L, D)
    start_row = (fidx * L).astype(np.int32)

    in_maps = []
    for c in range(NCORES):
        b0, b1 = c * NB, (c + 1) * NB
        in_maps.append({
            "hidden": hidden[b0:b1],
            "audio_pad": audio_pad[b0:b1],
            "start_row": start_row[b0:b1].reshape(1, NB),
            "wq": wq_ship, "wk": wk_ship, "wv": wv_ship, "wo": wo_ship,
            "bq": bq_ship, "bk": bk_ship, "bo": bo_ship,
        })
    return in_maps


def run(in_maps, **kwargs):
    nc = get_program()
    return run_bass_kernel_spmd(nc, in_maps, list(range(NCORES)), **kwargs)


def kernel(**inputs) -> np.ndarray:
    in_maps = prep_inputs(**inputs)
    res = run(in_maps)
    outs = [res.results[c]["out"] for c in range(NCORES)]
    return np.concatenate(outs, axis=0).astype(np.float32)
